# revision 2
# baseline (speedup 1.0000x reference)
"""CrossCondGPT2 forward on 8 trn2 NeuronCores — zero-collective version.

Sharding: pure data parallel over batch B=4; cores 2b and 2b+1 both compute
batch element b fully (redundant pair; no cross-core communication at all —
the per-layer pair collective of the previous design costs ~25ms/call under
this runtime and dominated exec time).

Per core, R=1024 rows (one batch element), fp32 residual h in SBUF:
  LN1 (bn_stats) -> a [R,C]; PE-transpose -> aT [C,R] f32r (natural order)
  V row-major from aT chunks -> V_aug [keys, 12, 65] bf16 (ones col for sums)
  per feature pair fo (6): qT[fo] (permuted q-tile cols [0,2,4,6,1,3,5,7]),
    kT[fo] (natural); then heads 2fo, 2fo+1:
      scoresT [k,1024]: even k-tiles both halves, odd k-tiles right half only
      (left=even q-tiles are fully masked vs odd k-tiles); exp without
      max-subtraction (|s| <= ~4 by construction; 1/8 folded into Wq);
      triangular mask via one [triu x4] mult per computed half; expT bf16
      AV: yT[65,1024] = V_aug^T @ expT per half (left: even kt only);
      divide by sums (row 64) via PE-broadcast of bf16 reciprocal
  proj row-major (lhsT = yT bf16 chunks, wp bf16), h += proj
  LN2 -> mT f32r; then per T-half: fc1 (w1 f32r streamed, gelu+bias on ACT
  evict) -> h1T [24][128,512] bf16; fc2 row-major (w2 bf16 streamed, 4 psum
  accumulator pairs), h += mlp

kernel(**inputs) takes FULL unsharded inputs, returns [B,T,C] fp32.
"""

import sys

if "/opt/trn_rl_repo" not in sys.path:
    sys.path.insert(0, "/opt/trn_rl_repo")

import numpy as np

import concourse.bacc as bacc
import concourse.mybir as mybir
import concourse.tile as tile

L, B, T, C, NH, HD, HID = 4, 4, 1024, 768, 12, 64, 3072
R = 1024  # rows per core (full batch element)
P = 128
NCORES = 8
F32, F32R = mybir.dt.float32, mybir.dt.float32r
BF16 = mybir.dt.bfloat16
AF = mybir.ActivationFunctionType
ALU = mybir.AluOpType
EPS = 1e-5

CT = C // P  # 6 feature chunks
RT = R // P  # 8 row tiles
KT = T // P  # 8 key tiles
HT = HID // P  # 24 hidden chunks

# permuted q-tile order: evens then odds; BPOS[g] = column block of natural tile g
QPERM = [0, 2, 4, 6, 1, 3, 5, 7]
BPOS = [QPERM.index(g) for g in range(RT)]


def build(zq, zp, z2, repeat=1):
    """zq/zp/z2: skip qkv / proj / fc2 bias paths when those biases are zero.

    repeat: wrap the whole layer stack in a For_i loop (timing builds only).
    """
    nc = bacc.Bacc(None, target_bir_lowering=False, debug=False)

    x_in = nc.declare_dram_parameter("x", [R, C], F32, isOutput=False)
    wq_in = nc.declare_dram_parameter("wq", [L, C, C], F32R, isOutput=False)
    wk_in = nc.declare_dram_parameter("wk", [L, C, C], F32R, isOutput=False)
    wv_in = nc.declare_dram_parameter("wv", [L, C, C], F32R, isOutput=False)
    wp_in = nc.declare_dram_parameter("wp", [L, C, C], BF16, isOutput=False)
    w1_in = nc.declare_dram_parameter("w1", [L, C, HID], F32R, isOutput=False)
    w2_in = nc.declare_dram_parameter("w2", [L, HID, C], BF16, isOutput=False)
    bq_in = nc.declare_dram_parameter("bq", [L, C], F32, isOutput=False)
    bk_in = nc.declare_dram_parameter("bk", [L, C], F32, isOutput=False)
    bv_in = nc.declare_dram_parameter("bv", [L, C], F32R, isOutput=False)
    bp_in = nc.declare_dram_parameter("bp", [L, C], BF16, isOutput=False)
    b1_in = nc.declare_dram_parameter("b1", [L, HID], F32, isOutput=False)
    b2_in = nc.declare_dram_parameter("b2", [L, C], BF16, isOutput=False)
    triu4_in = nc.declare_dram_parameter("triu4", [P, 4 * P], BF16, isOutput=False)
    ident_in = nc.declare_dram_parameter("ident", [P, P], F32, isOutput=False)
    ones_in = nc.declare_dram_parameter("ones_row", [1, P], F32R, isOutput=False)
    onesb_in = nc.declare_dram_parameter("ones_b", [1, P], BF16, isOutput=False)
    out_d = nc.declare_dram_parameter("out", [R, C], F32, isOutput=True)

    with tile.TileContext(nc) as tc:
        with (
            tc.tile_pool(name="res", bufs=1) as res,
            tc.tile_pool(name="act", bufs=6) as act_p,  # aT/mT [P,R] f32r
            tc.tile_pool(name="qk", bufs=7) as qk_p,  # qT x6 + kT rotating
            tc.tile_pool(name="yt", bufs=6) as yt_p,  # yT [P,R] bf16
            tc.tile_pool(name="et", bufs=8) as et_p,  # expT [P,R] bf16
            tc.tile_pool(name="h1", bufs=24) as h1_p,  # h1T [P,512] bf16
            tc.tile_pool(name="vaug", bufs=8) as vaug_p,
            tc.tile_pool(name="wqkv", bufs=7) as wqkv_p,
            tc.tile_pool(name="wpp", bufs=6) as wp_p,
            tc.tile_pool(name="w1p", bufs=8) as w1_p,
            tc.tile_pool(name="w2p", bufs=4) as w2_p,
            tc.tile_pool(name="rot", bufs=2) as rot,
            tc.tile_pool(name="small", bufs=2) as small,
            tc.tile_pool(name="ps", bufs=8, space="PSUM") as ps,
        ):
            # ---- persistent tiles ----
            ident = res.tile([P, P], F32, tag="ident")
            nc.sync.dma_start(ident[:], ident_in[:])
            triu4 = res.tile([P, 4 * P], BF16, tag="triu4")
            nc.sync.dma_start(triu4[:], triu4_in[:])
            ones_row = res.tile([1, P], F32R, tag="ones_row")
            nc.sync.dma_start(ones_row[:], ones_in[:])
            ones_b = res.tile([1, P], BF16, tag="ones_b")
            nc.sync.dma_start(ones_b[:], onesb_in[:])
            eps_t = res.tile([P, 1], F32, tag="eps")
            nc.vector.memset(eps_t[:], EPS)
            ones_pp = res.tile([P, NH], BF16, tag="ones_pp")
            nc.vector.memset(ones_pp[:], 1.0)

            h = []
            for rt in range(RT):
                ht_ = res.tile([P, C], F32, tag=f"h{rt}")
                nc.sync.dma_start(ht_[:], x_in[rt * P : (rt + 1) * P, :])
                h.append(ht_)

            def layernorm(rt):
                """Row-major LN of h[rt] (gain/bias folded downstream)."""
                a_t = rot.tile([P, C], F32, tag="ln_out", bufs=2)
                stats = small.tile([P, 3, 6], F32, tag="bn_stats")
                xg = h[rt][:].rearrange("p (g d) -> p g d", g=3)
                for g in range(3):
                    nc.vector.bn_stats(stats[:, g, :], xg[:, g, :])
                mv = small.tile([P, 2], F32, tag="bn_mv")
                nc.vector.bn_aggr(mv[:], stats[:])
                std = small.tile([P, 1], F32, tag="bn_std")
                nc.scalar.activation(
                    std[:], mv[:, 1:2], AF.Sqrt, bias=eps_t[:], scale=1.0
                )
                rstd = small.tile([P, 1], F32, tag="bn_rstd")
                nc.vector.reciprocal(rstd[:], std[:])
                nc.vector.tensor_scalar(
                    out=a_t[:],
                    in0=h[rt][:],
                    scalar1=mv[:, 0:1],
                    scalar2=rstd[:],
                    op0=ALU.subtract,
                    op1=ALU.mult,
                )
                return a_t

            def transpose_to_feat(tag):
                """LN all 8 row tiles -> 6 f32r feature-major [P, R] tiles."""
                ft = [
                    act_p.tile([P, R], F32R, tag="act", name=f"{tag}{fc}")
                    for fc in range(CT)
                ]
                for rt in range(RT):
                    a_t = layernorm(rt)
                    for fc in range(CT):
                        tp = ps.tile([P, P], F32, tag="ps")
                        nc.tensor.transpose(
                            tp[:], a_t[:, fc * P : (fc + 1) * P], ident[:]
                        )
                        nc.scalar.activation(
                            ft[fc][:, rt * P : (rt + 1) * P], tp[:], AF.Copy
                        )
                return ft

            def body(layer):
                lsl = slice(layer, layer + 1)

                # ---- qkv weight loads in use order: wv, wq, wk ----
                wv = [wqkv_p.tile([P, C], F32R, tag="wqkv", name=f"wv{i}") for i in range(CT)]
                for i in range(CT):
                    nc.sync.dma_start(wv[i][:], wv_in[layer, i * P : (i + 1) * P, :])
                wq = [wqkv_p.tile([P, C], F32R, tag="wqkv", name=f"wq{i}") for i in range(CT)]
                for i in range(CT):
                    nc.sync.dma_start(wq[i][:], wq_in[layer, i * P : (i + 1) * P, :])
                wk = [wqkv_p.tile([P, C], F32R, tag="wqkv", name=f"wk{i}") for i in range(CT)]
                for i in range(CT):
                    nc.sync.dma_start(wk[i][:], wk_in[layer, i * P : (i + 1) * P, :])

                if not zq:
                    bq_sb = small.tile([P, CT], F32, tag="bq_sb")
                    bk_sb = small.tile([P, CT], F32, tag="bk_sb")
                    bv_row = small.tile([1, C], F32R, tag="bv_row")
                    nc.sync.dma_start(
                        bq_sb[:], bq_in[lsl, :].rearrange("o (f p) -> p (o f)", p=P)
                    )
                    nc.sync.dma_start(
                        bk_sb[:], bk_in[lsl, :].rearrange("o (f p) -> p (o f)", p=P)
                    )
                    nc.sync.dma_start(bv_row[:], bv_in[lsl, :])

                # ---- LN1 + transpose ----
                aT = transpose_to_feat("aT")

                # ---- V row-major -> V_aug [keys, 12, 65] bf16, ones col ----
                v_aug = [
                    vaug_p.tile([P, NH, HD + 1], BF16, tag="vaug", name=f"va{k}")
                    for k in range(KT)
                ]
                for kt in range(KT):
                    ksl = slice(kt * P, (kt + 1) * P)
                    pv8 = ps.tile([P, 512], F32, tag="ps")
                    pv4 = ps.tile([P, 256], F32, tag="ps")
                    if not zq:
                        nc.tensor.matmul(
                            pv8[:], ones_row[:], bv_row[:, 0:512], start=True, stop=False
                        )
                        nc.tensor.matmul(
                            pv4[:], ones_row[:], bv_row[:, 512:768], start=True, stop=False
                        )
                    for i in range(CT):
                        nc.tensor.matmul(
                            pv8[:],
                            aT[i][:, ksl],
                            wv[i][:, 0:512],
                            start=(zq and i == 0),
                            stop=(i == CT - 1),
                        )
                        nc.tensor.matmul(
                            pv4[:],
                            aT[i][:, ksl],
                            wv[i][:, 512:768],
                            start=(zq and i == 0),
                            stop=(i == CT - 1),
                        )
                    nc.vector.tensor_copy(
                        v_aug[kt][:, 0:8, 0:HD],
                        pv8[:].rearrange("p (h d) -> p h d", d=HD),
                    )
                    nc.vector.tensor_copy(
                        v_aug[kt][:, 8:12, 0:HD],
                        pv4[:].rearrange("p (h d) -> p h d", d=HD),
                    )
                    nc.vector.tensor_copy(
                        v_aug[kt][:, :, HD : HD + 1].rearrange("p h o -> p (h o)"),
                        ones_pp[:],
                    )

                # ---- qT for all fo (wq resident), permuted q-tile columns ----
                yT = [
                    yt_p.tile([P, R], BF16, tag="yt", name=f"yT{i}")
                    for i in range(CT)
                ]
                qTs = []
                for fo in range(CT):
                    qT = qk_p.tile([P, R], F32R, tag="qk", name=f"qT{fo}")
                    pq0 = ps.tile([P, 512], F32, tag="ps")
                    pq1 = ps.tile([P, 512], F32, tag="ps")
                    for i in range(CT):
                        for hf, pq in ((0, pq0), (1, pq1)):
                            nc.tensor.matmul(
                                pq[:],
                                wq[i][:, fo * P : (fo + 1) * P],
                                aT[i][:, hf * 512 : (hf + 1) * 512],
                                start=(i == 0),
                                stop=(i == CT - 1),
                            )
                    for hf, pq in ((0, pq0), (1, pq1)):
                        # scatter natural tiles 4*hf+j into permuted slots
                        for j in range(4):
                            g = 4 * hf + j
                            dst = qT[:, BPOS[g] * P : (BPOS[g] + 1) * P]
                            src = pq[:, j * P : (j + 1) * P]
                            if zq:
                                nc.scalar.activation(dst, src, AF.Copy)
                            else:
                                nc.scalar.activation(
                                    dst, src, AF.Identity, bias=bq_sb[:, fo : fo + 1]
                                )
                    qTs.append(qT)

                # ---- per feature-pair: kT then two heads ----
                for fo in range(CT):
                    qT = qTs[fo]
                    # kT[fo]: natural key order
                    kTf = qk_p.tile([P, R], F32R, tag="qk", name=f"kT{fo}")
                    pk0 = ps.tile([P, 512], F32, tag="ps")
                    pk1 = ps.tile([P, 512], F32, tag="ps")
                    for i in range(CT):
                        for hf, pk in ((0, pk0), (1, pk1)):
                            nc.tensor.matmul(
                                pk[:],
                                wk[i][:, fo * P : (fo + 1) * P],
                                aT[i][:, hf * 512 : (hf + 1) * 512],
                                start=(i == 0),
                                stop=(i == CT - 1),
                            )
                    for hf, pk in ((0, pk0), (1, pk1)):
                        dst = kTf[:, hf * 512 : (hf + 1) * 512]
                        if zq:
                            nc.scalar.activation(dst, pk[:], AF.Copy)
                        else:
                            nc.scalar.activation(
                                dst, pk[:], AF.Identity, bias=bk_sb[:, fo : fo + 1]
                            )

                    for hh in (2 * fo, 2 * fo + 1):
                        psl = slice((hh % 2) * HD, (hh % 2) * HD + HD)
                        expT = []
                        for kt in range(KT):
                            et = et_p.tile([P, R], BF16, tag="et", name=f"et{kt}")
                            if kt % 2 == 0:
                                for hf in range(2):
                                    pscr = ps.tile([P, 512], F32, tag="ps")
                                    nc.tensor.matmul(
                                        pscr[:],
                                        kTf[psl, kt * P : (kt + 1) * P],
                                        qT[psl, hf * 512 : (hf + 1) * 512],
                                        start=True,
                                        stop=True,
                                    )
                                    sl_ = slice(hf * 512, (hf + 1) * 512)
                                    nc.scalar.activation(et[:, sl_], pscr[:], AF.Exp)
                                # left half (even q-tiles): triangular mask
                                nc.vector.tensor_tensor(
                                    out=et[:, 0:512],
                                    in0=et[:, 0:512],
                                    in1=triu4[:],
                                    op=ALU.mult,
                                )
                            else:
                                pscr = ps.tile([P, 512], F32, tag="ps")
                                nc.tensor.matmul(
                                    pscr[:],
                                    kTf[psl, kt * P : (kt + 1) * P],
                                    qT[psl, 512:1024],
                                    start=True,
                                    stop=True,
                                )
                                nc.scalar.activation(
                                    et[:, 512:1024], pscr[:], AF.Exp
                                )
                                # right half (odd q-tiles): triangular mask
                                nc.vector.tensor_tensor(
                                    out=et[:, 512:1024],
                                    in0=et[:, 512:1024],
                                    in1=triu4[:],
                                    op=ALU.mult,
                                )
                            expT.append(et)

                        # AV: py[65, 1024] = V_aug^T @ expT (left: even kt only)
                        pyL = ps.tile([P, 512], F32, tag="ps")
                        pyR = ps.tile([P, 512], F32, tag="ps")
                        for j in range(KT):
                            if j % 2 == 0:
                                nc.tensor.matmul(
                                    pyL[: HD + 1, :],
                                    v_aug[j][:, hh, :],
                                    expT[j][:, 0:512],
                                    start=(j == 0),
                                    stop=(j == KT - 2),
                                )
                            nc.tensor.matmul(
                                pyR[: HD + 1, :],
                                v_aug[j][:, hh, :],
                                expT[j][:, 512:1024],
                                start=(j == 0),
                                stop=(j == KT - 1),
                            )
                        # divide by sums (row HD) via PE broadcast of recip
                        rrow = small.tile([1, R], BF16, tag="rrow", bufs=1)
                        with nc.allow_low_precision(reason="bf16 recip for bcast"):
                            nc.vector.reciprocal(rrow[:, 0:512], pyL[HD : HD + 1, :])
                            nc.vector.reciprocal(rrow[:, 512:1024], pyR[HD : HD + 1, :])
                        for hf, pyh in ((0, pyL), (1, pyR)):
                            pb = ps.tile([HD, 512], F32, tag="ps")
                            nc.tensor.matmul(
                                pb[:],
                                ones_b[:, :HD],
                                rrow[:, hf * 512 : (hf + 1) * 512],
                                start=True,
                                stop=True,
                            )
                            sbb = small.tile([HD, 512], F32, tag="sbb")
                            nc.scalar.activation(sbb[:], pb[:], AF.Copy)
                            nc.vector.tensor_tensor(
                                out=yT[fo][psl, hf * 512 : (hf + 1) * 512],
                                in0=pyh[:HD, :],
                                in1=sbb[:],
                                op=ALU.mult,
                            )

                # ---- proj + residual (row-major; un-permute via BPOS) ----
                wp = [wp_p.tile([P, C], BF16, tag="wp", name=f"wp{i}") for i in range(CT)]
                for i in range(CT):
                    nc.sync.dma_start(wp[i][:], wp_in[layer, i * P : (i + 1) * P, :])
                if not zp:
                    bp_row = small.tile([1, C], BF16, tag="bp_row")
                    nc.sync.dma_start(bp_row[:], bp_in[lsl, :])
                for g in range(RT):
                    tsl = slice(BPOS[g] * P, (BPOS[g] + 1) * P)
                    ppA = ps.tile([P, 512], F32, tag="ps")
                    ppB = ps.tile([P, 256], F32, tag="ps")
                    if not zp:
                        nc.tensor.matmul(
                            ppA[:], ones_b[:], bp_row[:, 0:512], start=True, stop=False
                        )
                        nc.tensor.matmul(
                            ppB[:], ones_b[:], bp_row[:, 512:768], start=True, stop=False
                        )
                    for i in range(CT):
                        nc.tensor.matmul(
                            ppA[:],
                            yT[i][:, tsl],
                            wp[i][:, 0:512],
                            start=(zp and i == 0),
                            stop=(i == CT - 1),
                        )
                        nc.tensor.matmul(
                            ppB[:],
                            yT[i][:, tsl],
                            wp[i][:, 512:768],
                            start=(zp and i == 0),
                            stop=(i == CT - 1),
                        )
                    nc.vector.tensor_tensor(
                        out=h[g][:, 0:512], in0=h[g][:, 0:512], in1=ppA[:], op=ALU.add
                    )
                    nc.vector.tensor_tensor(
                        out=h[g][:, 512:768], in0=h[g][:, 512:768], in1=ppB[:], op=ALU.add
                    )

                # ---- MLP ----
                mT = transpose_to_feat("mT")

                b1_sb = small.tile([P, HT], F32, tag="b1_sb")
                nc.sync.dma_start(
                    b1_sb[:], b1_in[lsl, :].rearrange("o (f p) -> p (o f)", p=P)
                )
                if not z2:
                    b2_row = small.tile([1, C], BF16, tag="b2_row")
                    nc.sync.dma_start(b2_row[:], b2_in[lsl, :])

                for hf in range(2):
                    # fc1 half: h1T [24][128,512] bf16
                    h1T = [
                        h1_p.tile([P, 512], BF16, tag="h1", name=f"h1T{i}")
                        for i in range(HT)
                    ]
                    for ofg in range(6):
                        w1c = [
                            w1_p.tile([P, 512], F32R, tag="w1c", name=f"w1c{i}")
                            for i in range(CT)
                        ]
                        for i in range(CT):
                            nc.sync.dma_start(
                                w1c[i][:],
                                w1_in[
                                    layer,
                                    i * P : (i + 1) * P,
                                    ofg * 512 : (ofg + 1) * 512,
                                ],
                            )
                        for oi in range(4):
                            of = ofg * 4 + oi
                            pf = ps.tile([P, 512], F32, tag="ps")
                            for i in range(CT):
                                nc.tensor.matmul(
                                    pf[:],
                                    w1c[i][:, oi * P : (oi + 1) * P],
                                    mT[i][:, hf * 512 : (hf + 1) * 512],
                                    start=(i == 0),
                                    stop=(i == CT - 1),
                                )
                            nc.scalar.activation(
                                h1T[of][:],
                                pf[:],
                                AF.Gelu,
                                bias=b1_sb[:, of : of + 1],
                            )

                    # fc2 half: 4 t-tiles (rows hf*512 ..), psum accumulators
                    pacc = []
                    for gi in range(4):
                        pa = ps.tile([P, 512], F32, tag="ps", name=f"pa{gi}")
                        pb_ = ps.tile([P, 256], F32, tag="ps", name=f"pb{gi}")
                        pacc.append((pa, pb_))
                        if not z2:
                            nc.tensor.matmul(
                                pa[:], ones_b[:], b2_row[:, 0:512], start=True, stop=False
                            )
                            nc.tensor.matmul(
                                pb_[:], ones_b[:], b2_row[:, 512:768], start=True, stop=False
                            )
                    w2 = [
                        w2_p.tile([P, C], BF16, tag="w2", name=f"w2_{i}")
                        for i in range(HT)
                    ]
                    for i in range(HT):
                        nc.sync.dma_start(
                            w2[i][:], w2_in[layer, i * P : (i + 1) * P, :]
                        )
                        for gi in range(4):
                            lhs = h1T[i][:, gi * P : (gi + 1) * P]
                            pa, pb_ = pacc[gi]
                            nc.tensor.matmul(
                                pa[:],
                                lhs,
                                w2[i][:, 0:512],
                                start=(z2 and i == 0),
                                stop=(i == HT - 1),
                            )
                            nc.tensor.matmul(
                                pb_[:],
                                lhs,
                                w2[i][:, 512:768],
                                start=(z2 and i == 0),
                                stop=(i == HT - 1),
                            )
                    for gi in range(4):
                        g = hf * 4 + gi
                        pa, pb_ = pacc[gi]
                        nc.vector.tensor_tensor(
                            out=h[g][:, 0:512], in0=h[g][:, 0:512], in1=pa[:], op=ALU.add
                        )
                        nc.vector.tensor_tensor(
                            out=h[g][:, 512:768], in0=h[g][:, 512:768], in1=pb_[:], op=ALU.add
                        )

            if repeat == 1:
                for layer in range(L):
                    body(layer)
            else:
                with tc.For_i(0, repeat) as _i:
                    for layer in range(L):
                        body(layer)

            for rt in range(RT):
                nc.sync.dma_start(out_d[rt * P : (rt + 1) * P, :], h[rt][:])

    nc.compile()
    return nc


# ------------------------ host side ------------------------

_CACHE = {}


def _prep_inputs(inputs):
    import ml_dtypes

    f32 = np.float32
    bf16 = ml_dtypes.bfloat16
    g1 = inputs["ln1_g"].astype(f32)[:, :, None]
    b1g = inputs["ln1_b"].astype(f32)
    g2 = inputs["ln2_g"].astype(f32)[:, :, None]
    b2g = inputs["ln2_b"].astype(f32)

    def fold(Wname, bname, g, b, scale=1.0):
        W = inputs[Wname].astype(f32)
        bias = inputs[bname].astype(f32)
        Weff = (g * W) * scale
        beff = (bias + np.einsum("lc,lcd->ld", b, W)) * scale
        return Weff.astype(f32), beff.astype(f32)

    wq, bq = fold("Wq", "bq", g1, b1g, 0.125)
    wk, bk = fold("Wk", "bk", g1, b1g)
    wv, bv = fold("Wv", "bv", g1, b1g)
    w1, b1 = fold("W1", "b1", g2, b2g)
    bp = inputs["bp"].astype(f32)
    b2 = inputs["b2"].astype(f32)

    triu = np.triu(np.ones((P, P), np.float32))
    common = {
        "wq": wq,
        "wk": wk,
        "wv": wv,
        "wp": inputs["Wp"].astype(bf16),
        "w1": w1,
        "w2": inputs["W2"].astype(bf16),
        "bq": bq,
        "bk": bk,
        "bv": bv,
        "bp": bp.astype(bf16),
        "b1": b1,
        "b2": b2.astype(bf16),
        "triu4": np.tile(triu, (1, 4)).astype(bf16),
        "ident": np.eye(P, dtype=np.float32),
        "ones_row": np.ones((1, P), np.float32),
        "ones_b": np.ones((1, P), bf16),
    }
    zq = bool(np.all(bq == 0) and np.all(bk == 0) and np.all(bv == 0))
    zp = bool(np.all(bp == 0))
    z2 = bool(np.all(b2 == 0))
    x = inputs["x"].astype(f32)
    shards = [np.ascontiguousarray(x[c // 2]) for c in range(NCORES)]
    return common, shards, (zq, zp, z2)


def get_nc(flags, repeat=1):
    key = (*flags, repeat)
    if key not in _CACHE:
        _CACHE[key] = build(*flags, repeat=repeat)
    return _CACHE[key]


def kernel(**inputs):
    from concourse.bass_utils import run_bass_kernel_spmd

    common, shards, flags = _prep_inputs(inputs)
    nc = get_nc(flags)
    in_maps = [dict(common, x=shards[c]) for c in range(NCORES)]
    res = run_bass_kernel_spmd(nc, in_maps, list(range(NCORES)), trace=False)
    out = np.empty((B, T, C), np.float32)
    for b in range(B):
        out[b] = res.results[2 * b]["out"]
    return out


if __name__ == "__main__":
    nc = build(True, True, True)
    print("build+compile OK")
